# revision 7
# baseline (speedup 1.0000x reference)
"""Causal multi-head attention on 8 TRN2 NeuronCores, collective edition.

Core = (batch b, head-group hg): b = core//2, hg = core%2 (6 of 12 heads).
Wire traffic is the bottleneck (~67 MB/s axon tunnel), so every input is
shipped exactly once across the 8 cores and gathered on device:
  xh    [384, 2048]  bf16  half of xT[b]      -> AllGather over pair {2b,2b+1}
  wqkvh [192, 1152]  bf16  quarter of wqkv_hg -> AllGather over quad {hg,hg+2,..}
  wph   [96, 768]    bf16  quarter of wp_hg   -> AllGather over quad
Output: partial yT [768,2048] bf16 ReduceScattered (add) over the pair; the
local [384, 2048] slice is int8-quantized with a per-token scale
c*sigma_t/127 (sigma_t from an exact PE partition-sum of squares), and shipped
as ONE packed tensor outq [388, 2048] int8 — rows 384:388 hold the f32 scale
row's bytes. Host dequantizes + stacks + transposes. Extra output tensors cost
~60ms each on this dispatch path, hence the packing; gpsimd partition ops cost
~100ms, hence PE/DVE-only epilogue.

Compute per core (same as validated baseline):
  qT,kT [384, 2048]  (head-major, head h at rows h*64..h*64+63)
  v [2048, 6, 65] per 128-row block; col 64 = 1.0 -> rowsum trick
  ST = kT_h[:, jblk].T @ qT_h[:, itile]; PT = exp(ST/8) causal via affine_select
  OT [65, 512] += v[jblk,h].T @ PT (row 64 = softmax denom); out = OT * 1/denom
  yT_partial = wp_hg.T @ oT
"""

import numpy as np
import ml_dtypes
import jax

# Each run_bass_kernel_spmd call re-jits its shard_map wrapper; the persistent
# compilation cache turns that ~0.27s/dispatch XLA recompile into a ~0.1s hit.
jax.config.update("jax_compilation_cache_dir", "/tmp/jax_comp_cache")
jax.config.update("jax_persistent_cache_min_compile_time_secs", 0.0)
jax.config.update("jax_persistent_cache_min_entry_size_bytes", 0)

QC = 4.0  # int8 quant range = QC * per-token sigma

B, N, C = 4, 2048, 768
H, D = 12, 64
HG = 6          # heads per core
CG = HG * D     # 384 local head channels
NCORES = 8
NB = N // 128   # 16 j-blocks
NT = N // 512   # 4 i-tiles
CCH = C // 128  # 6 contraction chunks
PAIRS = [[0, 1], [2, 3], [4, 5], [6, 7]]
QUADS = [[0, 2, 4, 6], [1, 3, 5, 7]]

_COMPILED = {}


def _build():
    import concourse.bass as bass
    import concourse.mybir as mybir
    import concourse.tile as tile
    from concourse import bacc

    fp32 = mybir.dt.float32
    bf16 = mybir.dt.bfloat16
    i8 = mybir.dt.int8
    Exp = mybir.ActivationFunctionType.Exp
    Square = mybir.ActivationFunctionType.Square
    Sqrt = mybir.ActivationFunctionType.Sqrt

    nc = bacc.Bacc(None, target_bir_lowering=False, num_devices=NCORES)
    xh = nc.declare_dram_parameter("xh", [CG, N], bf16, isOutput=False)
    wqkvh = nc.declare_dram_parameter("wqkvh", [192, 3 * CG], bf16, isOutput=False)
    wph = nc.declare_dram_parameter("wph", [96, C], bf16, isOutput=False)
    outq = nc.declare_dram_parameter("outq", [CG + 4, N], i8, isOutput=True)

    with tile.TileContext(nc) as tc:
        with (
            tc.tile_pool(name="dram", bufs=1, space="DRAM") as dp,
            tc.tile_pool(name="persist", bufs=1) as pp,
            tc.tile_pool(name="work", bufs=3) as wkp,
            tc.tile_pool(name="outp", bufs=3) as op,
            tc.tile_pool(name="ps_mm", bufs=2, space="PSUM") as ps_mm,
            tc.tile_pool(name="ps_st", bufs=3, space="PSUM") as ps_st,
            tc.tile_pool(name="ps_ot", bufs=2, space="PSUM") as ps_ot,
        ):
            # ---- gather sharded inputs on device ----
            xh_b = dp.tile([CG, N], bf16, name="xh_b")
            xg = dp.tile([C, N], bf16, name="xg")
            wqkv_b = dp.tile([192, 3 * CG], bf16, name="wqkv_b")
            wqkv_g = dp.tile([C, 3 * CG], bf16, name="wqkv_g")
            wp_b = dp.tile([96, C], bf16, name="wp_b")
            wp_g = dp.tile([CG, C], bf16, name="wp_g")
            y_b = dp.tile([C, N], bf16, name="y_b")
            y_r = dp.tile([CG, N], bf16, name="y_r")

            nc.gpsimd.dma_start(xh_b[:], xh[:])
            nc.gpsimd.dma_start(wqkv_b[:], wqkvh[:])
            nc.gpsimd.dma_start(wp_b[:], wph[:])
            nc.gpsimd.collective_compute(
                "AllGather", mybir.AluOpType.bypass, PAIRS,
                ins=[xh_b[:].opt()], outs=[xg[:].opt()])
            nc.gpsimd.collective_compute(
                "AllGather", mybir.AluOpType.bypass, QUADS,
                ins=[wqkv_b[:].opt()], outs=[wqkv_g[:].opt()])
            nc.gpsimd.collective_compute(
                "AllGather", mybir.AluOpType.bypass, QUADS,
                ins=[wp_b[:].opt()], outs=[wp_g[:].opt()])

            # ---- load gathered inputs to SBUF ----
            xT_sb = [pp.tile([128, N], bf16, name=f"xT{i}") for i in range(CCH)]
            wqkv_sb = [pp.tile([128, 3 * CG], bf16, name=f"wqkv{i}")
                       for i in range(CCH)]
            wp_sb = [pp.tile([128, C], bf16, name=f"wp{i}") for i in range(3)]
            for i in range(CCH):
                nc.sync.dma_start(xT_sb[i][:], xg[i * 128:(i + 1) * 128, :])
                nc.sync.dma_start(wqkv_sb[i][:], wqkv_g[i * 128:(i + 1) * 128, :])
            for i in range(3):
                nc.sync.dma_start(wp_sb[i][:], wp_g[i * 128:(i + 1) * 128, :])

            qT_sb = [pp.tile([128, N], bf16, name=f"qT{g}") for g in range(3)]
            kT_sb = [pp.tile([128, N], bf16, name=f"kT{g}") for g in range(3)]
            v_sb = [pp.tile([128, HG, 65], bf16, name=f"v{nb}") for nb in range(NB)]
            oT_sb = [pp.tile([128, N], bf16, name=f"oT{g}") for g in range(3)]

            # ---- qT / kT : [384, 2048] = w.T @ xT ----
            for dst, off in ((qT_sb, 0), (kT_sb, CG)):
                for g in range(3):
                    for nt in range(NT):
                        ps = ps_mm.tile([128, 512], fp32, name="ps_qk", tag="ps")
                        for ci in range(CCH):
                            nc.tensor.matmul(
                                ps[:],
                                lhsT=wqkv_sb[ci][:, off + g * 128:off + (g + 1) * 128],
                                rhs=xT_sb[ci][:, nt * 512:(nt + 1) * 512],
                                start=(ci == 0), stop=(ci == CCH - 1),
                            )
                        nc.any.tensor_copy(
                            out=dst[g][:, nt * 512:(nt + 1) * 512], in_=ps[:])

            # ---- v : per 128-row block [128, 6, 65], ones in col 64 ----
            for nb in range(NB):
                ps = ps_mm.tile([128, 512], fp32, name="ps_v", tag="ps")[:, :CG]
                for ci in range(CCH):
                    nc.tensor.matmul(
                        ps[:],
                        lhsT=xT_sb[ci][:, nb * 128:(nb + 1) * 128],
                        rhs=wqkv_sb[ci][:, 2 * CG:3 * CG],
                        start=(ci == 0), stop=(ci == CCH - 1),
                    )
                nc.vector.memset(v_sb[nb][:, :, 64], 1.0)
                nc.any.tensor_copy(
                    out=v_sb[nb][:, :, 0:64],
                    in_=ps[:].rearrange("p (h d) -> p h d", d=64),
                )

            # ---- attention per head ----
            scale = float(D) ** -0.5
            for h in range(HG):
                g, ro = h // 2, (h % 2) * 64
                for it in range(NT):
                    jmax = 4 * it + 3
                    ot = ps_ot.tile([65, 512], fp32, name="ps_ot")
                    for jb in range(jmax + 1):
                        st = ps_st.tile([128, 512], fp32, name="ps_st")
                        nc.tensor.matmul(
                            st[:],
                            lhsT=kT_sb[g][ro:ro + 64, jb * 128:(jb + 1) * 128],
                            rhs=qT_sb[g][ro:ro + 64, it * 512:(it + 1) * 512],
                            start=True, stop=True,
                        )
                        pt = wkp.tile([128, 512], bf16, name="pt", tag="pt")
                        nc.scalar.activation(pt[:], st[:], Exp, scale=scale)
                        if jb >= 4 * it:  # diagonal block: zero j > i
                            nc.gpsimd.affine_select(
                                out=pt[:], in_=pt[:],
                                pattern=[[1, 512]],
                                compare_op=mybir.AluOpType.is_ge,
                                fill=0.0,
                                base=it * 512 - jb * 128,
                                channel_multiplier=-1,
                            )
                        nc.tensor.matmul(
                            ot[:],
                            lhsT=v_sb[jb][:, h, :],
                            rhs=pt[:],
                            start=(jb == 0), stop=(jb == jmax),
                        )
                    rec = wkp.tile([1, 512], fp32, name="rec", tag="rec")
                    nc.vector.reciprocal(rec[:], ot[64:65, :])
                    rec64 = wkp.tile([64, 512], fp32, name="rec64", tag="rec64")
                    nc.gpsimd.partition_broadcast(rec64[:], rec[:])
                    nc.vector.tensor_tensor(
                        oT_sb[g][ro:ro + 64, it * 512:(it + 1) * 512],
                        ot[0:64, :],
                        rec64[:],
                        mybir.AluOpType.mult,
                    )

            # ---- proj: yT_partial [768, 2048] = wp.T @ oT -> DRAM bounce ----
            for g in range(6):
                for nt in range(NT):
                    ps = ps_mm.tile([128, 512], fp32, name="ps_y", tag="ps")
                    for ci in range(3):
                        nc.tensor.matmul(
                            ps[:],
                            lhsT=wp_sb[ci][:, g * 128:(g + 1) * 128],
                            rhs=oT_sb[ci][:, nt * 512:(nt + 1) * 512],
                            start=(ci == 0), stop=(ci == 2),
                        )
                    yt = op.tile([128, 512], bf16, name="yt", tag="yt")
                    nc.any.tensor_copy(out=yt[:], in_=ps[:])
                    nc.sync.dma_start(
                        y_b[g * 128:(g + 1) * 128, nt * 512:(nt + 1) * 512],
                        yt[:])

            # ---- pair-reduce: each core keeps a disjoint [384, 2048] slice ----
            nc.gpsimd.collective_compute(
                "ReduceScatter", mybir.AluOpType.add, PAIRS,
                ins=[y_b[:].opt()], outs=[y_r[:].opt()])

            # ---- int8 per-token quantization (PE/DVE only) ----
            ones_col = pp.tile([128, 1], bf16, name="ones_col")
            ones_row = pp.tile([1, 128], fp32, name="ones_row")
            nc.vector.memset(ones_col[:], 1.0)
            nc.vector.memset(ones_row[:], 1.0)
            ys_sb = [op.tile([128, N], bf16, name=f"ys{i}") for i in range(3)]
            for i in range(3):
                nc.sync.dma_start(ys_sb[i][:], y_r[i * 128:(i + 1) * 128, :])
            # scale row = QC*sigma_t/127 = sqrt(sum_c y^2 * QC^2/(127^2*CG))
            alpha = QC * QC / (127.0 * 127.0 * CG)
            sc_row = pp.tile([1, N], fp32, name="sc_row")
            sinv_row = pp.tile([1, N], fp32, name="sinv_row")
            for nt in range(NT):
                psS = ps_mm.tile([1, 512], fp32, name="ps_eS", tag="ps")
                for i in range(3):
                    ysq = wkp.tile([128, 512], bf16, name="ysq", tag="ysq")
                    nc.scalar.activation(
                        ysq[:], ys_sb[i][:, nt * 512:(nt + 1) * 512], Square)
                    nc.tensor.matmul(
                        psS[:], lhsT=ones_col[:], rhs=ysq[:],
                        start=(i == 0), stop=(i == 2))
                nc.scalar.activation(
                    sc_row[:, nt * 512:(nt + 1) * 512], psS[:], Sqrt,
                    scale=alpha)
            nc.vector.reciprocal(sinv_row[:], sc_row[:])
            qt_sb = [op.tile([128, N], i8, name=f"qt{i}") for i in range(3)]
            for nt in range(NT):
                psB = ps_st.tile([128, 512], fp32, name="ps_st")
                nc.tensor.matmul(
                    psB[:], lhsT=ones_row[:],
                    rhs=sinv_row[:, nt * 512:(nt + 1) * 512],
                    start=True, stop=True)
                for i in range(3):
                    prod = wkp.tile([128, 512], fp32, name="prod", tag="prod")
                    nc.vector.tensor_tensor(
                        prod[:], ys_sb[i][:, nt * 512:(nt + 1) * 512], psB[:],
                        mybir.AluOpType.mult)
                    nc.vector.tensor_scalar(
                        out=qt_sb[i][:, nt * 512:(nt + 1) * 512], in0=prod[:],
                        scalar1=-127.0, scalar2=127.0,
                        op0=mybir.AluOpType.max, op1=mybir.AluOpType.min)
            for i in range(3):
                nc.sync.dma_start(outq[i * 128:(i + 1) * 128, :], qt_sb[i][:])
            sc_i8 = sc_row[:].bitcast(i8)
            for r in range(4):
                nc.sync.dma_start(
                    outq[CG + r:CG + r + 1, :],
                    sc_i8[:, r * N:(r + 1) * N])
    nc.compile()
    return nc


def _make_in_maps(x, w_qkv, w_proj):
    bf = ml_dtypes.bfloat16
    x = np.asarray(x, np.float32)
    w_qkv = np.asarray(w_qkv, np.float32)
    w_proj = np.asarray(w_proj, np.float32)
    wq_f, wk_f, wv_f = w_qkv[:, :C], w_qkv[:, C:2 * C], w_qkv[:, 2 * C:]
    wqkv_hg, wp_hg, xT = [], [], []
    for hg in range(2):
        cs = slice(hg * CG, (hg + 1) * CG)
        wqkv_hg.append(np.concatenate(
            [wq_f[:, cs], wk_f[:, cs], wv_f[:, cs]], axis=1).astype(bf))
        wp_hg.append(w_proj[cs, :].astype(bf))
    for b in range(B):
        xT.append(np.ascontiguousarray(x[b].T).astype(bf))
    in_maps = []
    for core in range(NCORES):
        b, hg = core // 2, core % 2
        in_maps.append({
            "xh": np.ascontiguousarray(xT[b][hg * CG:(hg + 1) * CG, :]),
            "wqkvh": np.ascontiguousarray(wqkv_hg[hg][b * 192:(b + 1) * 192, :]),
            "wph": np.ascontiguousarray(wp_hg[hg][b * 96:(b + 1) * 96, :]),
        })
    return in_maps


def _dispatch(nc, in_maps):
    """run_bass_kernel_spmd with retries: the axon tunnel occasionally drops
    ("worker hung up"); clearing jax backends re-dials it on the next call."""
    import time
    from concourse.bass_utils import run_bass_kernel_spmd

    last = None
    for attempt in range(4):
        try:
            return run_bass_kernel_spmd(nc, in_maps, core_ids=list(range(NCORES)))
        except Exception as e:  # noqa: BLE001
            last = e
            for reset in (lambda: jax.clear_caches(),
                          lambda: jax.extend.backend.clear_backends()):
                try:
                    reset()
                except Exception:  # noqa: BLE001
                    pass
            time.sleep(2.0 * (attempt + 1))
    raise last


def kernel(x, w_qkv, w_proj, b_proj):
    if "nc" not in _COMPILED:
        _COMPILED["nc"] = _build()
    nc = _COMPILED["nc"]

    b_proj = np.asarray(b_proj, np.float32)
    in_maps = _make_in_maps(x, w_qkv, w_proj)
    res = _dispatch(nc, in_maps)
    y = np.empty((B, N, C), np.float32)
    for b in range(B):
        parts = []
        for r in (res.results[2 * b], res.results[2 * b + 1]):
            raw = np.asarray(r["outq"])
            sc = np.frombuffer(raw[CG:CG + 4].tobytes(), np.float32)
            parts.append(raw[:CG].astype(np.float32) * sc[None, :])
        y[b] = np.concatenate(parts, axis=0).T
    y += b_proj[None, None, :]
    return y


# revision 8
# speedup vs baseline: 1.1210x; 1.1210x over previous
"""Causal multi-head attention on 8 TRN2 NeuronCores, collective edition.

Core = (batch b, head-group hg): b = core//2, hg = core%2 (6 of 12 heads).
Wire traffic is the bottleneck (~67 MB/s axon tunnel), so every input is
shipped exactly once across the 8 cores and gathered on device:
  xh    [1152, 1024] uint8 12-bit-packed half of xT[b] (3 byte-planes; value =
        round(x/s[c]) + 2048, s folded into w_qkv rows on host)
                                            -> AllGather over pair {2b,2b+1}
  wqkvh [192, 1152]  bf16  quarter of s-scaled wqkv_hg -> AllGather over quad
  wph   [96, 768]    bf16  quarter of wp_hg            -> AllGather over quad
Output: partial yT [768,2048] bf16 ReduceScattered (add) over the pair; the
local [384, 2048] slice is int8-quantized with a per-token scale
c*sigma_t/127 (sigma_t from an exact PE partition-sum of squares), and shipped
as ONE packed tensor outq [388, 2048] int8 — rows 384:388 hold the f32 scale
row's bytes. Host dequantizes + stacks + transposes. Extra output tensors cost
~60ms each on this dispatch path, hence the packing; gpsimd partition ops cost
~100ms, hence PE/DVE-only epilogue.

Compute per core (same as validated baseline):
  qT,kT [384, 2048]  (head-major, head h at rows h*64..h*64+63)
  v [2048, 6, 65] per 128-row block; col 64 = 1.0 -> rowsum trick
  ST = kT_h[:, jblk].T @ qT_h[:, itile]; PT = exp(ST/8) causal via affine_select
  OT [65, 512] += v[jblk,h].T @ PT (row 64 = softmax denom); out = OT * 1/denom
  yT_partial = wp_hg.T @ oT
"""

import numpy as np
import ml_dtypes
import jax

# Each run_bass_kernel_spmd call re-jits its shard_map wrapper; the persistent
# compilation cache turns that ~0.27s/dispatch XLA recompile into a ~0.1s hit.
jax.config.update("jax_compilation_cache_dir", "/tmp/jax_comp_cache")
jax.config.update("jax_persistent_cache_min_compile_time_secs", 0.0)
jax.config.update("jax_persistent_cache_min_entry_size_bytes", 0)

QC = 4.0  # int8 quant range = QC * per-token sigma

B, N, C = 4, 2048, 768
H, D = 12, 64
HG = 6          # heads per core
CG = HG * D     # 384 local head channels
NCORES = 8
NB = N // 128   # 16 j-blocks
NT = N // 512   # 4 i-tiles
CCH = C // 128  # 6 contraction chunks
PAIRS = [[0, 1], [2, 3], [4, 5], [6, 7]]
QUADS = [[0, 2, 4, 6], [1, 3, 5, 7]]

_COMPILED = {}


def _build():
    import concourse.bass as bass
    import concourse.mybir as mybir
    import concourse.tile as tile
    from concourse import bacc

    fp32 = mybir.dt.float32
    bf16 = mybir.dt.bfloat16
    i8 = mybir.dt.int8
    Exp = mybir.ActivationFunctionType.Exp
    Square = mybir.ActivationFunctionType.Square
    Sqrt = mybir.ActivationFunctionType.Sqrt

    u8 = mybir.dt.uint8
    nc = bacc.Bacc(None, target_bir_lowering=False, num_devices=NCORES)
    # x shipped as 12-bit packed planes: rows 0:384 = u0 (low byte of even-half
    # value a), 384:768 = u1 (a>>8 | (b&15)<<4), 768:1152 = u2 (b>>4), where
    # a = tokens 0:1024, b = tokens 1024:2048, values = round(x/s[c]) + 2048.
    xh = nc.declare_dram_parameter("xh", [3 * CG, N // 2], u8, isOutput=False)
    wqkvh = nc.declare_dram_parameter("wqkvh", [192, 3 * CG], bf16, isOutput=False)
    wph = nc.declare_dram_parameter("wph", [96, C], bf16, isOutput=False)
    outq = nc.declare_dram_parameter("outq", [CG + 4, N], i8, isOutput=True)

    with tile.TileContext(nc) as tc:
        with (
            tc.tile_pool(name="dram", bufs=1, space="DRAM") as dp,
            tc.tile_pool(name="persist", bufs=1) as pp,
            tc.tile_pool(name="work", bufs=3) as wkp,
            tc.tile_pool(name="outp", bufs=3) as op,
            tc.tile_pool(name="ps_mm", bufs=2, space="PSUM") as ps_mm,
            tc.tile_pool(name="ps_st", bufs=3, space="PSUM") as ps_st,
            tc.tile_pool(name="ps_ot", bufs=2, space="PSUM") as ps_ot,
        ):
            # ---- gather sharded inputs on device ----
            xh_b = dp.tile([3 * CG, N // 2], u8, name="xh_b")
            xg = dp.tile([2 * 3 * CG, N // 2], u8, name="xg")
            wqkv_b = dp.tile([192, 3 * CG], bf16, name="wqkv_b")
            wqkv_g = dp.tile([C, 3 * CG], bf16, name="wqkv_g")
            wp_b = dp.tile([96, C], bf16, name="wp_b")
            wp_g = dp.tile([CG, C], bf16, name="wp_g")
            y_b = dp.tile([C, N], bf16, name="y_b")
            y_r = dp.tile([CG, N], bf16, name="y_r")

            nc.gpsimd.dma_start(xh_b[:], xh[:])
            nc.gpsimd.dma_start(wqkv_b[:], wqkvh[:])
            nc.gpsimd.dma_start(wp_b[:], wph[:])
            nc.gpsimd.collective_compute(
                "AllGather", mybir.AluOpType.bypass, PAIRS,
                ins=[xh_b[:].opt()], outs=[xg[:].opt()])
            nc.gpsimd.collective_compute(
                "AllGather", mybir.AluOpType.bypass, QUADS,
                ins=[wqkv_b[:].opt()], outs=[wqkv_g[:].opt()])
            nc.gpsimd.collective_compute(
                "AllGather", mybir.AluOpType.bypass, QUADS,
                ins=[wp_b[:].opt()], outs=[wp_g[:].opt()])

            # ---- load gathered inputs to SBUF (x: unpack 12-bit -> bf16) ----
            xT_sb = [pp.tile([128, N], bf16, name=f"xT{i}") for i in range(CCH)]
            wqkv_sb = [pp.tile([128, 3 * CG], bf16, name=f"wqkv{i}")
                       for i in range(CCH)]
            wp_sb = [pp.tile([128, C], bf16, name=f"wp{i}") for i in range(3)]
            NH = N // 2
            for i in range(CCH):
                h, lr = i // 3, (i % 3) * 128
                base = h * 3 * CG + lr
                up = [wkp.tile([128, NH], u8, name=f"up{p}", tag=f"up{p}",
                               bufs=1) for p in range(3)]
                for p in range(3):
                    nc.sync.dma_start(
                        up[p][:], xg[base + p * CG:base + p * CG + 128, :])
                # hi4 = u1 >> 4, exactly: round(u1/16 - 0.46875) (no ties)
                hi4 = wkp.tile([128, NH], i8, name="hi4", tag="hi4", bufs=1)
                nc.vector.tensor_scalar(
                    out=hi4[:], in0=up[1][:], scalar1=1.0 / 16.0,
                    scalar2=-0.46875,
                    op0=mybir.AluOpType.mult, op1=mybir.AluOpType.add)
                # a = u0 + 256*u1 - 4096*hi4 - 2048  (tokens 0:1024)
                t1 = wkp.tile([128, NH], fp32, name="t1", tag="t1", bufs=1)
                nc.vector.tensor_scalar(
                    out=t1[:], in0=up[1][:], scalar1=256.0, scalar2=-2048.0,
                    op0=mybir.AluOpType.mult, op1=mybir.AluOpType.add)
                nc.vector.tensor_tensor(
                    t1[:], t1[:], up[0][:], mybir.AluOpType.add)
                t3 = wkp.tile([128, NH], fp32, name="t3", tag="t3", bufs=1)
                nc.vector.tensor_scalar(
                    out=t3[:], in0=hi4[:], scalar1=-4096.0, scalar2=None,
                    op0=mybir.AluOpType.mult)
                nc.vector.tensor_tensor(
                    xT_sb[i][:, 0:NH], t1[:], t3[:], mybir.AluOpType.add)
                # b = 16*u2 + hi4 - 2048  (tokens 1024:2048)
                t4 = wkp.tile([128, NH], fp32, name="t4", tag="t4", bufs=1)
                nc.vector.tensor_scalar(
                    out=t4[:], in0=up[2][:], scalar1=16.0, scalar2=-2048.0,
                    op0=mybir.AluOpType.mult, op1=mybir.AluOpType.add)
                nc.vector.tensor_tensor(
                    xT_sb[i][:, NH:N], t4[:], hi4[:], mybir.AluOpType.add)
                nc.sync.dma_start(wqkv_sb[i][:], wqkv_g[i * 128:(i + 1) * 128, :])
            for i in range(3):
                nc.sync.dma_start(wp_sb[i][:], wp_g[i * 128:(i + 1) * 128, :])

            qT_sb = [pp.tile([128, N], bf16, name=f"qT{g}") for g in range(3)]
            kT_sb = [pp.tile([128, N], bf16, name=f"kT{g}") for g in range(3)]
            v_sb = [pp.tile([128, HG, 65], bf16, name=f"v{nb}") for nb in range(NB)]
            oT_sb = [pp.tile([128, N], bf16, name=f"oT{g}") for g in range(3)]

            # ---- qT / kT : [384, 2048] = w.T @ xT ----
            for dst, off in ((qT_sb, 0), (kT_sb, CG)):
                for g in range(3):
                    for nt in range(NT):
                        ps = ps_mm.tile([128, 512], fp32, name="ps_qk", tag="ps")
                        for ci in range(CCH):
                            nc.tensor.matmul(
                                ps[:],
                                lhsT=wqkv_sb[ci][:, off + g * 128:off + (g + 1) * 128],
                                rhs=xT_sb[ci][:, nt * 512:(nt + 1) * 512],
                                start=(ci == 0), stop=(ci == CCH - 1),
                            )
                        nc.any.tensor_copy(
                            out=dst[g][:, nt * 512:(nt + 1) * 512], in_=ps[:])

            # ---- v : per 128-row block [128, 6, 65], ones in col 64 ----
            for nb in range(NB):
                ps = ps_mm.tile([128, 512], fp32, name="ps_v", tag="ps")[:, :CG]
                for ci in range(CCH):
                    nc.tensor.matmul(
                        ps[:],
                        lhsT=xT_sb[ci][:, nb * 128:(nb + 1) * 128],
                        rhs=wqkv_sb[ci][:, 2 * CG:3 * CG],
                        start=(ci == 0), stop=(ci == CCH - 1),
                    )
                nc.vector.memset(v_sb[nb][:, :, 64], 1.0)
                nc.any.tensor_copy(
                    out=v_sb[nb][:, :, 0:64],
                    in_=ps[:].rearrange("p (h d) -> p h d", d=64),
                )

            # ---- attention per head ----
            scale = float(D) ** -0.5
            for h in range(HG):
                g, ro = h // 2, (h % 2) * 64
                for it in range(NT):
                    jmax = 4 * it + 3
                    ot = ps_ot.tile([65, 512], fp32, name="ps_ot")
                    for jb in range(jmax + 1):
                        st = ps_st.tile([128, 512], fp32, name="ps_st")
                        nc.tensor.matmul(
                            st[:],
                            lhsT=kT_sb[g][ro:ro + 64, jb * 128:(jb + 1) * 128],
                            rhs=qT_sb[g][ro:ro + 64, it * 512:(it + 1) * 512],
                            start=True, stop=True,
                        )
                        pt = wkp.tile([128, 512], bf16, name="pt", tag="pt")
                        nc.scalar.activation(pt[:], st[:], Exp, scale=scale)
                        if jb >= 4 * it:  # diagonal block: zero j > i
                            nc.gpsimd.affine_select(
                                out=pt[:], in_=pt[:],
                                pattern=[[1, 512]],
                                compare_op=mybir.AluOpType.is_ge,
                                fill=0.0,
                                base=it * 512 - jb * 128,
                                channel_multiplier=-1,
                            )
                        nc.tensor.matmul(
                            ot[:],
                            lhsT=v_sb[jb][:, h, :],
                            rhs=pt[:],
                            start=(jb == 0), stop=(jb == jmax),
                        )
                    rec = wkp.tile([1, 512], fp32, name="rec", tag="rec")
                    nc.vector.reciprocal(rec[:], ot[64:65, :])
                    rec64 = wkp.tile([64, 512], fp32, name="rec64", tag="rec64")
                    nc.gpsimd.partition_broadcast(rec64[:], rec[:])
                    nc.vector.tensor_tensor(
                        oT_sb[g][ro:ro + 64, it * 512:(it + 1) * 512],
                        ot[0:64, :],
                        rec64[:],
                        mybir.AluOpType.mult,
                    )

            # ---- proj: yT_partial [768, 2048] = wp.T @ oT -> DRAM bounce ----
            for g in range(6):
                for nt in range(NT):
                    ps = ps_mm.tile([128, 512], fp32, name="ps_y", tag="ps")
                    for ci in range(3):
                        nc.tensor.matmul(
                            ps[:],
                            lhsT=wp_sb[ci][:, g * 128:(g + 1) * 128],
                            rhs=oT_sb[ci][:, nt * 512:(nt + 1) * 512],
                            start=(ci == 0), stop=(ci == 2),
                        )
                    yt = op.tile([128, 512], bf16, name="yt", tag="yt")
                    nc.any.tensor_copy(out=yt[:], in_=ps[:])
                    nc.sync.dma_start(
                        y_b[g * 128:(g + 1) * 128, nt * 512:(nt + 1) * 512],
                        yt[:])

            # ---- pair-reduce: each core keeps a disjoint [384, 2048] slice ----
            nc.gpsimd.collective_compute(
                "ReduceScatter", mybir.AluOpType.add, PAIRS,
                ins=[y_b[:].opt()], outs=[y_r[:].opt()])

            # ---- int8 per-token quantization (PE/DVE only) ----
            ones_col = pp.tile([128, 1], bf16, name="ones_col")
            ones_row = pp.tile([1, 128], fp32, name="ones_row")
            nc.vector.memset(ones_col[:], 1.0)
            nc.vector.memset(ones_row[:], 1.0)
            ys_sb = [pp.tile([128, N], bf16, name=f"ys{i}") for i in range(3)]
            for i in range(3):
                nc.sync.dma_start(ys_sb[i][:], y_r[i * 128:(i + 1) * 128, :])
            # scale row = QC*sigma_t/127 = sqrt(sum_c y^2 * QC^2/(127^2*CG))
            alpha = QC * QC / (127.0 * 127.0 * CG)
            sc_row = pp.tile([1, N], fp32, name="sc_row")
            sinv_row = pp.tile([1, N], fp32, name="sinv_row")
            for nt in range(NT):
                psS = ps_mm.tile([1, 512], fp32, name="ps_eS", tag="ps")
                for i in range(3):
                    ysq = wkp.tile([128, 512], bf16, name="ysq", tag="ysq")
                    nc.scalar.activation(
                        ysq[:], ys_sb[i][:, nt * 512:(nt + 1) * 512], Square)
                    nc.tensor.matmul(
                        psS[:], lhsT=ones_col[:], rhs=ysq[:],
                        start=(i == 0), stop=(i == 2))
                nc.scalar.activation(
                    sc_row[:, nt * 512:(nt + 1) * 512], psS[:], Sqrt,
                    scale=alpha)
            nc.vector.reciprocal(sinv_row[:], sc_row[:])
            qt_sb = [pp.tile([128, N], i8, name=f"qt{i}") for i in range(3)]
            for nt in range(NT):
                psB = ps_st.tile([128, 512], fp32, name="ps_st")
                nc.tensor.matmul(
                    psB[:], lhsT=ones_row[:],
                    rhs=sinv_row[:, nt * 512:(nt + 1) * 512],
                    start=True, stop=True)
                for i in range(3):
                    prod = wkp.tile([128, 512], fp32, name="prod", tag="prod")
                    nc.vector.tensor_tensor(
                        prod[:], ys_sb[i][:, nt * 512:(nt + 1) * 512], psB[:],
                        mybir.AluOpType.mult)
                    nc.vector.tensor_scalar(
                        out=qt_sb[i][:, nt * 512:(nt + 1) * 512], in0=prod[:],
                        scalar1=-127.0, scalar2=127.0,
                        op0=mybir.AluOpType.max, op1=mybir.AluOpType.min)
            for i in range(3):
                nc.sync.dma_start(outq[i * 128:(i + 1) * 128, :], qt_sb[i][:])
            sc_i8 = sc_row[:].bitcast(i8)
            for r in range(4):
                nc.sync.dma_start(
                    outq[CG + r:CG + r + 1, :],
                    sc_i8[:, r * N:(r + 1) * N])
    nc.compile()
    return nc


def _make_in_maps(x, w_qkv, w_proj):
    bf = ml_dtypes.bfloat16
    x = np.asarray(x, np.float32)
    w_qkv = np.asarray(w_qkv, np.float32)
    w_proj = np.asarray(w_proj, np.float32)
    # 12-bit symmetric per-channel x quantization; scales fold into w_qkv rows
    s = np.maximum(np.abs(x).max(axis=(0, 1)) / 2047.0, 1e-20)  # [C]
    wq_s = w_qkv * s[:, None]
    wq_f, wk_f, wv_f = wq_s[:, :C], wq_s[:, C:2 * C], wq_s[:, 2 * C:]
    wqkv_hg, wp_hg = [], []
    for hg in range(2):
        cs = slice(hg * CG, (hg + 1) * CG)
        wqkv_hg.append(np.concatenate(
            [wq_f[:, cs], wk_f[:, cs], wv_f[:, cs]], axis=1).astype(bf))
        wp_hg.append(w_proj[cs, :].astype(bf))
    xq = (np.rint(x / s[None, None, :]).clip(-2047, 2047)
          .astype(np.int32) + 2048)
    NH = N // 2
    xpk = []
    for b in range(B):
        xT = xq[b].T  # [C, N] int32
        packed = np.empty((2, 3 * CG, NH), np.uint8)
        for hg in range(2):
            xh = xT[hg * CG:(hg + 1) * CG, :]
            a, bb = xh[:, :NH], xh[:, NH:]
            packed[hg, 0:CG] = a & 255
            packed[hg, CG:2 * CG] = (a >> 8) | ((bb & 15) << 4)
            packed[hg, 2 * CG:3 * CG] = bb >> 4
        xpk.append(packed)
    in_maps = []
    for core in range(NCORES):
        b, hg = core // 2, core % 2
        in_maps.append({
            "xh": np.ascontiguousarray(xpk[b][hg]),
            "wqkvh": np.ascontiguousarray(wqkv_hg[hg][b * 192:(b + 1) * 192, :]),
            "wph": np.ascontiguousarray(wp_hg[hg][b * 96:(b + 1) * 96, :]),
        })
    return in_maps


def _dispatch(nc, in_maps):
    """run_bass_kernel_spmd with retries: the axon tunnel occasionally drops
    ("worker hung up"); clearing jax backends re-dials it on the next call."""
    import time
    from concourse.bass_utils import run_bass_kernel_spmd

    last = None
    for attempt in range(4):
        try:
            return run_bass_kernel_spmd(nc, in_maps, core_ids=list(range(NCORES)))
        except Exception as e:  # noqa: BLE001
            last = e
            for reset in (lambda: jax.clear_caches(),
                          lambda: jax.extend.backend.clear_backends()):
                try:
                    reset()
                except Exception:  # noqa: BLE001
                    pass
            time.sleep(2.0 * (attempt + 1))
    raise last


def kernel(x, w_qkv, w_proj, b_proj):
    if "nc" not in _COMPILED:
        _COMPILED["nc"] = _build()
    nc = _COMPILED["nc"]

    b_proj = np.asarray(b_proj, np.float32)
    in_maps = _make_in_maps(x, w_qkv, w_proj)
    res = _dispatch(nc, in_maps)
    y = np.empty((B, N, C), np.float32)
    for b in range(B):
        parts = []
        for r in (res.results[2 * b], res.results[2 * b + 1]):
            raw = np.asarray(r["outq"])
            sc = np.frombuffer(raw[CG:CG + 4].tobytes(), np.float32)
            parts.append(raw[:CG].astype(np.float32) * sc[None, :])
        y[b] = np.concatenate(parts, axis=0).T
    y += b_proj[None, None, :]
    return y


# revision 9
# speedup vs baseline: 1.1889x; 1.0606x over previous
"""Causal multi-head attention on 8 TRN2 NeuronCores, collective edition.

Core = (batch b, head-group hg): b = core//2, hg = core%2 (6 of 12 heads).
Wire traffic is the bottleneck (~67 MB/s axon tunnel), so every input is
shipped exactly once across the 8 cores and gathered on device:
  xh    [1152, 1024] uint8 12-bit-packed half of xT[b] (3 byte-planes; value =
        round(x/s[c]) + 2048, s folded into w_qkv rows on host)
                                            -> AllGather over pair {2b,2b+1}
  wqkvh [192, 1152]  bf16  quarter of s-scaled wqkv_hg -> AllGather over quad
  wph   [96, 768]    bf16  quarter of wp_hg            -> AllGather over quad
Output: partial yT [768,2048] bf16 ReduceScattered (add) over the pair; the
local [384, 2048] slice is int8-quantized with a per-token scale
c*sigma_t/127 (sigma_t from an exact PE partition-sum of squares), and shipped
as ONE packed tensor outq [388, 2048] int8 — rows 384:388 hold the f32 scale
row's bytes. Host dequantizes + stacks + transposes. Extra output tensors cost
~60ms each on this dispatch path, hence the packing; gpsimd partition ops cost
~100ms, hence PE/DVE-only epilogue.

Compute per core (same as validated baseline):
  qT,kT [384, 2048]  (head-major, head h at rows h*64..h*64+63)
  v [2048, 6, 65] per 128-row block; col 64 = 1.0 -> rowsum trick
  ST = kT_h[:, jblk].T @ qT_h[:, itile]; PT = exp(ST/8) causal via affine_select
  OT [65, 512] += v[jblk,h].T @ PT (row 64 = softmax denom); out = OT * 1/denom
  yT_partial = wp_hg.T @ oT
"""

import numpy as np
import ml_dtypes
import jax

# Each run_bass_kernel_spmd call re-jits its shard_map wrapper; the persistent
# compilation cache turns that ~0.27s/dispatch XLA recompile into a ~0.1s hit.
jax.config.update("jax_compilation_cache_dir", "/tmp/jax_comp_cache")
jax.config.update("jax_persistent_cache_min_compile_time_secs", 0.0)
jax.config.update("jax_persistent_cache_min_entry_size_bytes", 0)

QC = 4.0  # int8 quant range = QC * per-token sigma

B, N, C = 4, 2048, 768
H, D = 12, 64
HG = 6          # heads per core
CG = HG * D     # 384 local head channels
NCORES = 8
NB = N // 128   # 16 j-blocks
NT = N // 512   # 4 i-tiles
CCH = C // 128  # 6 contraction chunks
PAIRS = [[0, 1], [2, 3], [4, 5], [6, 7]]
QUADS = [[0, 2, 4, 6], [1, 3, 5, 7]]

_COMPILED = {}


def _build():
    import concourse.bass as bass
    import concourse.mybir as mybir
    import concourse.tile as tile
    from concourse import bacc

    fp32 = mybir.dt.float32
    bf16 = mybir.dt.bfloat16
    i8 = mybir.dt.int8
    Exp = mybir.ActivationFunctionType.Exp
    Square = mybir.ActivationFunctionType.Square
    Sqrt = mybir.ActivationFunctionType.Sqrt

    u8 = mybir.dt.uint8
    nc = bacc.Bacc(None, target_bir_lowering=False, num_devices=NCORES)
    # x shipped as 10-bit packed planes over token quarters: value =
    # round(x/s[c]) + 512 in [0,1023]; plane p<4 = low byte of quarter p,
    # plane 4 = the four 2-bit highs: h0 | h1<<2 | h2<<4 | h3<<6.
    xh = nc.declare_dram_parameter("xh", [5 * CG, N // 4], u8, isOutput=False)
    wqkvh = nc.declare_dram_parameter("wqkvh", [192, 3 * CG], bf16, isOutput=False)
    wph = nc.declare_dram_parameter("wph", [96, C], bf16, isOutput=False)
    outq = nc.declare_dram_parameter("outq", [CG + 4, N], i8, isOutput=True)

    with tile.TileContext(nc) as tc:
        with (
            tc.tile_pool(name="dram", bufs=1, space="DRAM") as dp,
            tc.tile_pool(name="persist", bufs=1) as pp,
            tc.tile_pool(name="work", bufs=3) as wkp,
            tc.tile_pool(name="outp", bufs=3) as op,
            tc.tile_pool(name="ps_mm", bufs=2, space="PSUM") as ps_mm,
            tc.tile_pool(name="ps_st", bufs=3, space="PSUM") as ps_st,
            tc.tile_pool(name="ps_ot", bufs=2, space="PSUM") as ps_ot,
        ):
            # ---- gather sharded inputs on device ----
            xh_b = dp.tile([5 * CG, N // 4], u8, name="xh_b")
            xg = dp.tile([2 * 5 * CG, N // 4], u8, name="xg")
            wqkv_b = dp.tile([192, 3 * CG], bf16, name="wqkv_b")
            wqkv_g = dp.tile([C, 3 * CG], bf16, name="wqkv_g")
            wp_b = dp.tile([96, C], bf16, name="wp_b")
            wp_g = dp.tile([CG, C], bf16, name="wp_g")
            y_b = dp.tile([C, N], bf16, name="y_b")
            y_r = dp.tile([CG, N], bf16, name="y_r")

            nc.gpsimd.dma_start(xh_b[:], xh[:])
            nc.gpsimd.dma_start(wqkv_b[:], wqkvh[:])
            nc.gpsimd.dma_start(wp_b[:], wph[:])
            nc.gpsimd.collective_compute(
                "AllGather", mybir.AluOpType.bypass, PAIRS,
                ins=[xh_b[:].opt()], outs=[xg[:].opt()])
            nc.gpsimd.collective_compute(
                "AllGather", mybir.AluOpType.bypass, QUADS,
                ins=[wqkv_b[:].opt()], outs=[wqkv_g[:].opt()])
            nc.gpsimd.collective_compute(
                "AllGather", mybir.AluOpType.bypass, QUADS,
                ins=[wp_b[:].opt()], outs=[wp_g[:].opt()])

            # ---- load gathered inputs to SBUF (x: unpack 12-bit -> bf16) ----
            xT_sb = [pp.tile([128, N], bf16, name=f"xT{i}") for i in range(CCH)]
            wqkv_sb = [pp.tile([128, 3 * CG], bf16, name=f"wqkv{i}")
                       for i in range(CCH)]
            wp_sb = [pp.tile([128, C], bf16, name=f"wp{i}") for i in range(3)]
            NQ = N // 4
            Flo = mybir.AluOpType.mult, mybir.AluOpType.add
            for i in range(CCH):
                h, lr = i // 3, (i % 3) * 128
                base = h * 5 * CG + lr
                up = [wkp.tile([128, NQ], u8, name=f"up{p}", tag=f"up{p}",
                               bufs=1) for p in range(5)]
                for p in range(5):
                    nc.sync.dma_start(
                        up[p][:], xg[base + p * CG:base + p * CG + 128, :])
                # q1 = b4>>2, q2 = b4>>4, q3 = b4>>6 via exact round-floors
                # (fractions are k/4, so round(z/4 - 0.375) == floor(z/4))
                qs = []
                src = up[4]
                for j in range(3):
                    q = wkp.tile([128, NQ], i8, name=f"q{j}", tag=f"q{j}",
                                 bufs=1)
                    nc.vector.tensor_scalar(
                        out=q[:], in0=src[:], scalar1=0.25, scalar2=-0.375,
                        op0=Flo[0], op1=Flo[1])
                    qs.append(q)
                    src = q
                # quarter p: v_p = b_p + 256*(hi_p) - 512 where
                # hi_0 = b4-4q1, hi_1 = q1-4q2, hi_2 = q2-4q3, hi_3 = q3
                his = [(up[4], qs[0]), (qs[0], qs[1]), (qs[1], qs[2]),
                       (qs[2], None)]
                for p in range(4):
                    lo, hi = his[p]
                    t1 = wkp.tile([128, NQ], fp32, name="t1", tag="t1", bufs=1)
                    nc.vector.tensor_scalar(
                        out=t1[:], in0=lo[:], scalar1=256.0, scalar2=-512.0,
                        op0=Flo[0], op1=Flo[1])
                    nc.vector.tensor_tensor(
                        t1[:], t1[:], up[p][:], mybir.AluOpType.add)
                    if hi is not None:
                        t3 = wkp.tile([128, NQ], fp32, name="t3", tag="t3",
                                      bufs=1)
                        nc.vector.tensor_scalar(
                            out=t3[:], in0=hi[:], scalar1=-1024.0,
                            scalar2=None, op0=mybir.AluOpType.mult)
                        nc.vector.tensor_tensor(
                            xT_sb[i][:, p * NQ:(p + 1) * NQ], t1[:], t3[:],
                            mybir.AluOpType.add)
                    else:
                        nc.any.tensor_copy(
                            out=xT_sb[i][:, p * NQ:(p + 1) * NQ], in_=t1[:])
                nc.sync.dma_start(wqkv_sb[i][:], wqkv_g[i * 128:(i + 1) * 128, :])
            for i in range(3):
                nc.sync.dma_start(wp_sb[i][:], wp_g[i * 128:(i + 1) * 128, :])

            qT_sb = [pp.tile([128, N], bf16, name=f"qT{g}") for g in range(3)]
            kT_sb = [pp.tile([128, N], bf16, name=f"kT{g}") for g in range(3)]
            v_sb = [pp.tile([128, HG, 65], bf16, name=f"v{nb}") for nb in range(NB)]
            oT_sb = [pp.tile([128, N], bf16, name=f"oT{g}") for g in range(3)]

            # ---- qT / kT : [384, 2048] = w.T @ xT ----
            for dst, off in ((qT_sb, 0), (kT_sb, CG)):
                for g in range(3):
                    for nt in range(NT):
                        ps = ps_mm.tile([128, 512], fp32, name="ps_qk", tag="ps")
                        for ci in range(CCH):
                            nc.tensor.matmul(
                                ps[:],
                                lhsT=wqkv_sb[ci][:, off + g * 128:off + (g + 1) * 128],
                                rhs=xT_sb[ci][:, nt * 512:(nt + 1) * 512],
                                start=(ci == 0), stop=(ci == CCH - 1),
                            )
                        nc.any.tensor_copy(
                            out=dst[g][:, nt * 512:(nt + 1) * 512], in_=ps[:])

            # ---- v : per 128-row block [128, 6, 65], ones in col 64 ----
            for nb in range(NB):
                ps = ps_mm.tile([128, 512], fp32, name="ps_v", tag="ps")[:, :CG]
                for ci in range(CCH):
                    nc.tensor.matmul(
                        ps[:],
                        lhsT=xT_sb[ci][:, nb * 128:(nb + 1) * 128],
                        rhs=wqkv_sb[ci][:, 2 * CG:3 * CG],
                        start=(ci == 0), stop=(ci == CCH - 1),
                    )
                nc.vector.memset(v_sb[nb][:, :, 64], 1.0)
                nc.any.tensor_copy(
                    out=v_sb[nb][:, :, 0:64],
                    in_=ps[:].rearrange("p (h d) -> p h d", d=64),
                )

            # ---- attention per head ----
            scale = float(D) ** -0.5
            for h in range(HG):
                g, ro = h // 2, (h % 2) * 64
                for it in range(NT):
                    jmax = 4 * it + 3
                    ot = ps_ot.tile([65, 512], fp32, name="ps_ot")
                    for jb in range(jmax + 1):
                        st = ps_st.tile([128, 512], fp32, name="ps_st")
                        nc.tensor.matmul(
                            st[:],
                            lhsT=kT_sb[g][ro:ro + 64, jb * 128:(jb + 1) * 128],
                            rhs=qT_sb[g][ro:ro + 64, it * 512:(it + 1) * 512],
                            start=True, stop=True,
                        )
                        pt = wkp.tile([128, 512], bf16, name="pt", tag="pt")
                        nc.scalar.activation(pt[:], st[:], Exp, scale=scale)
                        if jb >= 4 * it:  # diagonal block: zero j > i
                            nc.gpsimd.affine_select(
                                out=pt[:], in_=pt[:],
                                pattern=[[1, 512]],
                                compare_op=mybir.AluOpType.is_ge,
                                fill=0.0,
                                base=it * 512 - jb * 128,
                                channel_multiplier=-1,
                            )
                        nc.tensor.matmul(
                            ot[:],
                            lhsT=v_sb[jb][:, h, :],
                            rhs=pt[:],
                            start=(jb == 0), stop=(jb == jmax),
                        )
                    rec = wkp.tile([1, 512], fp32, name="rec", tag="rec")
                    nc.vector.reciprocal(rec[:], ot[64:65, :])
                    rec64 = wkp.tile([64, 512], fp32, name="rec64", tag="rec64")
                    nc.gpsimd.partition_broadcast(rec64[:], rec[:])
                    nc.vector.tensor_tensor(
                        oT_sb[g][ro:ro + 64, it * 512:(it + 1) * 512],
                        ot[0:64, :],
                        rec64[:],
                        mybir.AluOpType.mult,
                    )

            # ---- proj: yT_partial [768, 2048] = wp.T @ oT -> DRAM bounce ----
            for g in range(6):
                for nt in range(NT):
                    ps = ps_mm.tile([128, 512], fp32, name="ps_y", tag="ps")
                    for ci in range(3):
                        nc.tensor.matmul(
                            ps[:],
                            lhsT=wp_sb[ci][:, g * 128:(g + 1) * 128],
                            rhs=oT_sb[ci][:, nt * 512:(nt + 1) * 512],
                            start=(ci == 0), stop=(ci == 2),
                        )
                    yt = op.tile([128, 512], bf16, name="yt", tag="yt")
                    nc.any.tensor_copy(out=yt[:], in_=ps[:])
                    nc.sync.dma_start(
                        y_b[g * 128:(g + 1) * 128, nt * 512:(nt + 1) * 512],
                        yt[:])

            # ---- pair-reduce: each core keeps a disjoint [384, 2048] slice ----
            nc.gpsimd.collective_compute(
                "ReduceScatter", mybir.AluOpType.add, PAIRS,
                ins=[y_b[:].opt()], outs=[y_r[:].opt()])

            # ---- int8 per-token quantization (PE/DVE only) ----
            ones_col = pp.tile([128, 1], bf16, name="ones_col")
            ones_row = pp.tile([1, 128], fp32, name="ones_row")
            nc.vector.memset(ones_col[:], 1.0)
            nc.vector.memset(ones_row[:], 1.0)
            ys_sb = [pp.tile([128, N], bf16, name=f"ys{i}") for i in range(3)]
            for i in range(3):
                nc.sync.dma_start(ys_sb[i][:], y_r[i * 128:(i + 1) * 128, :])
            # scale row = QC*sigma_t/127 = sqrt(sum_c y^2 * QC^2/(127^2*CG))
            alpha = QC * QC / (127.0 * 127.0 * CG)
            sc_row = pp.tile([1, N], fp32, name="sc_row")
            sinv_row = pp.tile([1, N], fp32, name="sinv_row")
            for nt in range(NT):
                psS = ps_mm.tile([1, 512], fp32, name="ps_eS", tag="ps")
                for i in range(3):
                    ysq = wkp.tile([128, 512], bf16, name="ysq", tag="ysq")
                    nc.scalar.activation(
                        ysq[:], ys_sb[i][:, nt * 512:(nt + 1) * 512], Square)
                    nc.tensor.matmul(
                        psS[:], lhsT=ones_col[:], rhs=ysq[:],
                        start=(i == 0), stop=(i == 2))
                nc.scalar.activation(
                    sc_row[:, nt * 512:(nt + 1) * 512], psS[:], Sqrt,
                    scale=alpha)
            nc.vector.reciprocal(sinv_row[:], sc_row[:])
            qt_sb = [pp.tile([128, N], i8, name=f"qt{i}") for i in range(3)]
            for nt in range(NT):
                psB = ps_st.tile([128, 512], fp32, name="ps_st")
                nc.tensor.matmul(
                    psB[:], lhsT=ones_row[:],
                    rhs=sinv_row[:, nt * 512:(nt + 1) * 512],
                    start=True, stop=True)
                for i in range(3):
                    prod = wkp.tile([128, 512], fp32, name="prod", tag="prod")
                    nc.vector.tensor_tensor(
                        prod[:], ys_sb[i][:, nt * 512:(nt + 1) * 512], psB[:],
                        mybir.AluOpType.mult)
                    nc.vector.tensor_scalar(
                        out=qt_sb[i][:, nt * 512:(nt + 1) * 512], in0=prod[:],
                        scalar1=-127.0, scalar2=127.0,
                        op0=mybir.AluOpType.max, op1=mybir.AluOpType.min)
            for i in range(3):
                nc.sync.dma_start(outq[i * 128:(i + 1) * 128, :], qt_sb[i][:])
            sc_i8 = sc_row[:].bitcast(i8)
            for r in range(4):
                nc.sync.dma_start(
                    outq[CG + r:CG + r + 1, :],
                    sc_i8[:, r * N:(r + 1) * N])
    nc.compile()
    return nc


def _make_in_maps(x, w_qkv, w_proj):
    bf = ml_dtypes.bfloat16
    x = np.asarray(x, np.float32)
    w_qkv = np.asarray(w_qkv, np.float32)
    w_proj = np.asarray(w_proj, np.float32)
    # 10-bit symmetric per-channel x quantization; scales fold into w_qkv rows
    s = np.maximum(np.abs(x).max(axis=(0, 1)) / 511.0, 1e-20)  # [C]
    wq_s = w_qkv * s[:, None]
    wq_f, wk_f, wv_f = wq_s[:, :C], wq_s[:, C:2 * C], wq_s[:, 2 * C:]
    wqkv_hg, wp_hg = [], []
    for hg in range(2):
        cs = slice(hg * CG, (hg + 1) * CG)
        wqkv_hg.append(np.concatenate(
            [wq_f[:, cs], wk_f[:, cs], wv_f[:, cs]], axis=1).astype(bf))
        wp_hg.append(w_proj[cs, :].astype(bf))
    xq = (np.rint(x / s[None, None, :]).clip(-511, 511)
          .astype(np.int32) + 512)
    NQ = N // 4
    xpk = []
    for b in range(B):
        xT = xq[b].T  # [C, N] int32
        packed = np.empty((2, 5 * CG, NQ), np.uint8)
        for hg in range(2):
            xh = xT[hg * CG:(hg + 1) * CG, :]
            v = [xh[:, p * NQ:(p + 1) * NQ] for p in range(4)]
            hi = np.zeros((CG, NQ), np.int32)
            for p in range(4):
                packed[hg, p * CG:(p + 1) * CG] = v[p] & 255
                hi |= (v[p] >> 8) << (2 * p)
            packed[hg, 4 * CG:5 * CG] = hi
        xpk.append(packed)
    in_maps = []
    for core in range(NCORES):
        b, hg = core // 2, core % 2
        in_maps.append({
            "xh": np.ascontiguousarray(xpk[b][hg]),
            "wqkvh": np.ascontiguousarray(wqkv_hg[hg][b * 192:(b + 1) * 192, :]),
            "wph": np.ascontiguousarray(wp_hg[hg][b * 96:(b + 1) * 96, :]),
        })
    return in_maps


def _dispatch(nc, in_maps):
    """run_bass_kernel_spmd with retries: the axon tunnel occasionally drops
    ("worker hung up"); clearing jax backends re-dials it on the next call."""
    import time
    from concourse.bass_utils import run_bass_kernel_spmd

    last = None
    for attempt in range(4):
        try:
            return run_bass_kernel_spmd(nc, in_maps, core_ids=list(range(NCORES)))
        except Exception as e:  # noqa: BLE001
            last = e
            for reset in (lambda: jax.clear_caches(),
                          lambda: jax.extend.backend.clear_backends()):
                try:
                    reset()
                except Exception:  # noqa: BLE001
                    pass
            time.sleep(2.0 * (attempt + 1))
    raise last


def kernel(x, w_qkv, w_proj, b_proj):
    if "nc" not in _COMPILED:
        _COMPILED["nc"] = _build()
    nc = _COMPILED["nc"]

    b_proj = np.asarray(b_proj, np.float32)
    in_maps = _make_in_maps(x, w_qkv, w_proj)
    res = _dispatch(nc, in_maps)
    y = np.empty((B, N, C), np.float32)
    for b in range(B):
        parts = []
        for r in (res.results[2 * b], res.results[2 * b + 1]):
            raw = np.asarray(r["outq"])
            sc = np.frombuffer(raw[CG:CG + 4].tobytes(), np.float32)
            parts.append(raw[:CG].astype(np.float32) * sc[None, :])
        y[b] = np.concatenate(parts, axis=0).T
    y += b_proj[None, None, :]
    return y
